# revision 33
# baseline (speedup 1.0000x reference)
"""DWHT (buggy in-place Walsh-Hadamard channel transform + channel shuffle) on 8 trn2 cores.

The whole nn.Module is a fixed linear map on the channel axis:
    y[b, :, h, w] = T @ x[b, :, h, w]
with T a (512, 256) matrix of small integers (|T| <= 13, exactly representable
in fp8-e4m3).  Batch 64 is sharded 8-ways (data parallel, 8 samples/core); each
core runs a tiled PE matmul: for every sample, y_s (512,784) = T @ x_s (256,784).

Precision strategy (variant "fp8dr", default): the checker gate is 2e-2
relative error, so 16-bit transfer precision is comfortably inside
tolerance.  The host wrapper rounds x to bf16 (rel err ~1e-3 RMS) and then
splits it EXACTLY into two fp8-e4m3 halves, x = hi + lo: hi is the e4m3
rounding of x (top 4 mantissa bits) and lo = x - hi is an integer multiple
of the bf16 ulp no larger than 8 ulps, hence also exact in e4m3.  T's small
integers are exact in e4m3 too, so the device computes T@hi + T@lo in two
fp8 DoubleRow matmuls (K=256 packed two-per-partition, 0.5 PE cycles/row --
half the PE time of the bf16 equivalent) accumulating exactly into fp32
PSUM.  Output is evicted PSUM->SBUF with a bf16 cast and DMA'd out as bf16;
the host upcasts to fp32.  Total error ~2e-3, all from the initial bf16
rounding of x and the final bf16 rounding of y.

HBM traffic per core: 3.2MB in (two e4m3 tensors) + 6.4MB out (bf16)
+ 0.26MB weights ~= 9.9MB, half the fp32-I/O figure.  Work is spread
across all five engine queues (PE matmuls + a few output DMAs; SP input
DMAs + output DMAs; Pool copies + output DMAs; DVE/ACT PSUM-evict copies)
so no single engine holds more than ~15us of the ~74us total.
"""

import os
import sys

import numpy as np

for _p in ("/opt/trn_rl_repo", "/root/.axon_site/_ro/trn_rl_repo"):
    if os.path.isdir(_p) and _p not in sys.path:
        sys.path.append(_p)

B, C_IN, C_OUT, HH, WW = 64, 256, 512, 28, 28
S = HH * WW  # 784
N_CORES = 8
BS = B // N_CORES  # 8 samples per core
N_PASSES, GROUPS = 8, 8

VARIANT = os.environ.get("DWHT_VARIANT", "fp8dr")

# spatial split per PSUM bank (each chunk <= 512 fp32 = one bank)
N_CHUNKS = ((0, 392), (392, 392))


def _dwht_T() -> np.ndarray:
    """Build the (512, 256) transform matrix by running the reference
    butterfly (including its partial-update in-place semantics) on identity."""
    x = np.zeros((C_OUT, C_IN), np.float64)
    x[:C_IN] = np.eye(C_IN)
    half = C_OUT // 2
    for _ in range(N_PASSES):
        top = x[::2] + x[1::2]
        x = x.copy()
        x[:half] = top
        bottom = x[::2] - x[1::2]
        x[half:] = bottom
    # channel shuffle with groups=8
    x = x.reshape(GROUPS, C_OUT // GROUPS, C_IN).transpose(1, 0, 2).reshape(C_OUT, C_IN)
    return x


def _plan(env, default):
    return os.environ.get(env, default).split(",")


def _interleave(counts):
    """Bresenham-interleave engine names by target counts -> flat list."""
    total = sum(counts.values())
    acc = {k: 0.0 for k in counts}
    out = []
    for i in range(total):
        k = max(counts, key=lambda k: counts[k] / total * (i + 1) - acc[k])
        acc[k] += 1
        out.append(k)
    return out


def _build(variant):
    import concourse.mybir as mybir
    from concourse import bacc
    from concourse.tile import TileContext

    f32 = mybir.dt.float32
    bf16 = mybir.dt.bfloat16
    f8 = mybir.dt.float8e4

    fp8 = variant == "fp8dr"
    perf_mode = mybir.MatmulPerfMode.DoubleRow if fp8 else None

    # Engine schedules.  Inputs are front-loaded on the two HWDGE queues so
    # the PE never starves; early outputs go to Pool/ACT (SP is busy with
    # inputs), late outputs to SP (inputs done); copies are spread
    # Pool-heavy (327ns each) with DVE next (533ns) and ACT last (552ns).
    def sched(env, default):
        p = _plan(env, default)
        return lambda i: p[i % len(p)]

    # inputs: xh on scalar, xl on sync (sync also carries the weights first)
    # -- both HWDGE queues feed in parallel the whole run, finishing ~5us in,
    # after which both take output DMAs.
    in_s = sched("DWHT_IN_PLAN", "scalar,sync")
    # full-tile outs (28, samples 0-6) alternate the two HWDGE queues with a
    # couple on Pool; Pool is otherwise the copy workhorse (327ns/copy vs
    # DVE 533 / ACT 552)
    out_s = sched("DWHT_OUT_PLAN",
                  "scalar,sync,scalar,sync,scalar,gpsimd,sync,scalar,sync,"
                  "scalar,sync,scalar,sync,scalar,sync,gpsimd,scalar,sync,"
                  "scalar,sync,scalar,sync,scalar,sync,scalar,sync,scalar,"
                  "gpsimd")
    # copies: DVE 7 / Pool 9 per 16 -> 28/36 over 64; final copy on Pool
    copy_s = sched("DWHT_COPY_PLAN",
                   "vector,gpsimd,gpsimd,vector,gpsimd,vector,gpsimd,vector,"
                   "gpsimd,gpsimd,vector,gpsimd,vector,gpsimd,vector,gpsimd")
    # last-sample drain: per-chunk DMAs fanned across both HWDGE queues
    tail_dma = sched("DWHT_TAIL_DMA", "scalar,sync")
    nwarm = int(os.environ.get("DWHT_WARM", "0"))
    prefetch = int(os.environ.get("DWHT_PREFETCH", "4"))
    xs_bufs = int(os.environ.get("DWHT_XS_BUFS", "12"))

    nc = bacc.Bacc(None, target_bir_lowering=False)
    if fp8:
        # host-packed [s, p, (t a), f]: t = hi/lo split half, a = k-half,
        # p = partition -- one contiguous 3136B run per partition per sample
        xhl = nc.dram_tensor("xhl", (BS, 128, 4, S), f8, kind="ExternalInput")
        tt = nc.dram_tensor("tt", (C_IN, C_OUT), f8, kind="ExternalInput")
    else:
        x = nc.dram_tensor("x", (BS, C_IN, S), bf16, kind="ExternalInput")
        tt = nc.dram_tensor("tt", (C_IN, C_OUT), bf16, kind="ExternalInput")
    y = nc.dram_tensor("y", (BS, C_OUT, S), bf16, kind="ExternalOutput")

    def eng(name):
        return getattr(nc, name)

    def copy_op(name, dst, src):
        if name == "scalar":
            nc.scalar.copy(dst, src)
        else:
            eng(name).tensor_copy(dst, src)

    with TileContext(nc) as tc:
        with (
            tc.tile_pool(name="w", bufs=1) as wp,
            tc.tile_pool(name="io", bufs=3) as io,
            tc.tile_pool(name="ps", bufs=8, space="PSUM") as pp,
        ):
            if fp8:
                # [128, 2, 512]: partition p, k-half a, out-channel m.
                # Split per m-tile so the first matmul's weights land ASAP
                # behind the fixed HWDGE pipe-init latency.
                tw = wp.tile([128, 2, C_OUT], f8, tag="tw")
                ttr = tt.rearrange("(a p) m -> p a m", p=128)
                nc.sync.dma_start(out=tw[:], in_=ttr)
                if nwarm:
                    # dummy matmuls on the weight tile to ramp the PE clock
                    # while the first input DMAs are in flight
                    warm = pp.tile([128, 392], f32, tag="ps", name="warm")
                    for wi in range(nwarm):
                        nc.tensor.matmul(
                            warm[:],
                            tw[:, :, 0:128],
                            tw[:, :, 0:392],
                            start=(wi == 0),
                            stop=(wi == nwarm - 1),
                            perf_mode=perf_mode,
                        )
            else:
                tts = []
                for k in range(2):
                    t = wp.tile([128, C_OUT], bf16, tag=f"tt{k}")
                    nc.sync.dma_start(out=t[:], in_=tt[k * 128 : (k + 1) * 128, :])
                    tts.append(t)

            n_out = n_cp = 0
            in_tiles = {}

            def issue_in(s):
                if fp8:
                    xt = io.tile([128, 4, S], f8, tag="xs", bufs=xs_bufs)
                    if s == 0:
                        # split by half and chunk across idle queues:
                        # minimize time-to-first-matmul behind DMA pipe-init
                        for ti, q in ((0, "scalar"), (1, "gpsimd")):
                            tsl = slice(2 * ti, 2 * ti + 2)
                            for n0, nsz in N_CHUNKS:
                                eng(q).dma_start(
                                    out=xt[:, tsl, n0 : n0 + nsz],
                                    in_=xhl[s][:, tsl, n0 : n0 + nsz],
                                )
                    else:
                        eng(in_s(s)).dma_start(out=xt[:], in_=xhl[s])
                    srcs = [xt[:, 0:2], xt[:, 2:4]]
                else:
                    srcs = []
                    for k in range(2):
                        xt = io.tile([128, S], bf16, tag="xs", bufs=xs_bufs)
                        eng(in_s(2 * s + k)).dma_start(
                            out=xt[:], in_=x[s, k * 128 : (k + 1) * 128, :]
                        )
                        srcs.append(xt)
                in_tiles[s] = srcs

            for s in range(min(prefetch, BS)):
                issue_in(s)

            for s in range(BS):
                last = s == BS - 1
                if s + prefetch < BS:
                    issue_in(s + prefetch)
                srcs = in_tiles.pop(s)

                for m in range(C_OUT // 128):
                    msl = slice(m * 128, (m + 1) * 128)
                    ysm = io.tile([128, S], bf16, tag="ysm", bufs=12,
                                  name="ysm")
                    for ni, (n0, nsz) in enumerate(N_CHUNKS):
                        nsl = slice(n0, n0 + nsz)
                        ps = pp.tile([128, nsz], f32, tag="ps")
                        if fp8:
                            for i, xt in enumerate(srcs):
                                nc.tensor.matmul(
                                    ps[:],
                                    tw[:, :, msl],
                                    xt[:, :, nsl],
                                    start=(i == 0),
                                    stop=(i == 1),
                                    perf_mode=perf_mode,
                                )
                        else:
                            for ki in range(2):
                                nc.tensor.matmul(
                                    ps[:],
                                    tts[ki][:, msl],
                                    srcs[ki][:, nsl],
                                    start=(ki == 0),
                                    stop=(ki == 1),
                                )
                        ci = m * len(N_CHUNKS) + ni
                        copy_op(copy_s(n_cp), ysm[:, nsl], ps[:])
                        n_cp += 1
                        if last:
                            # drain per-chunk on fanned-out queues
                            eng(tail_dma(ci)).dma_start(
                                out=y[s, msl, nsl], in_=ysm[:, nsl]
                            )
                    if not last:
                        eng(out_s(n_out)).dma_start(
                            out=y[s, msl, :], in_=ysm[:]
                        )
                    n_out += 1

    nc.compile()
    return nc


_cache = {}


def _get_nc(variant, reps=1):
    key = (
        variant,
        os.environ.get("DWHT_IN_PLAN"),
        os.environ.get("DWHT_OUT_PLAN"),
        os.environ.get("DWHT_COPY_PLAN"),
    )
    if key not in _cache:
        _cache[key] = _build(variant)
    return _cache[key]


def _in_maps(x_np, variant):
    import ml_dtypes

    T = _dwht_T()
    ttT = np.ascontiguousarray(T.T)  # (256, 512), lhsT layout
    if variant == "fp8dr":
        tt_np = ttT.astype(ml_dtypes.float8_e4m3fn)
        x_bf = x_np.astype(ml_dtypes.bfloat16).astype(np.float32)
        x_hi = x_bf.astype(ml_dtypes.float8_e4m3fn)
        x_lo = (x_bf - x_hi.astype(np.float32)).astype(ml_dtypes.float8_e4m3fn)
        # pack [b, p, (t a), f]: j=0 hi k<128, j=1 hi k>=128, j=2/3 lo halves
        xhl = np.empty((B, 128, 4, S), ml_dtypes.float8_e4m3fn)
        xhl[:, :, 0] = x_hi[:, 0:128]
        xhl[:, :, 1] = x_hi[:, 128:256]
        xhl[:, :, 2] = x_lo[:, 0:128]
        xhl[:, :, 3] = x_lo[:, 128:256]
        return [
            {"xhl": xhl[i * BS : (i + 1) * BS], "tt": tt_np}
            for i in range(N_CORES)
        ]
    tt_np = ttT.astype(ml_dtypes.bfloat16)
    x_bf = x_np.astype(ml_dtypes.bfloat16)
    return [
        {"x": x_bf[i * BS : (i + 1) * BS], "tt": tt_np} for i in range(N_CORES)
    ]


def _run(x_np, variant=None, trace=False, reps=1):
    from concourse.bass_utils import run_bass_kernel_spmd

    variant = variant or VARIANT
    nc = _get_nc(variant, reps)
    res = run_bass_kernel_spmd(
        nc, _in_maps(x_np, variant), list(range(N_CORES)), trace=trace
    )
    y = (
        np.stack([np.asarray(r["y"], np.float32) for r in res.results])
        .reshape(B, C_OUT, HH, WW)
    )
    return y, res


def kernel(x: np.ndarray) -> np.ndarray:
    x_np = np.ascontiguousarray(np.asarray(x), dtype=np.float32).reshape(B, C_IN, S)
    y, _ = _run(x_np)
    return y


# revision 34
# speedup vs baseline: 1.1811x; 1.1811x over previous
"""DWHT (buggy in-place Walsh-Hadamard channel transform + channel shuffle) on 8 trn2 cores.

The whole nn.Module is a fixed linear map on the channel axis:
    y[b, :, h, w] = T @ x[b, :, h, w]
with T a (512, 256) matrix of small integers (|T| <= 13, exactly representable
in fp8-e4m3).  Batch 64 is sharded 8-ways (data parallel, 8 samples/core); each
core runs a tiled PE matmul: for every sample, y_s (512,784) = T @ x_s (256,784).

Precision strategy (variant "fp8dr", default): the checker gate is 2e-2
relative error, so 16-bit transfer precision is comfortably inside
tolerance.  The host wrapper rounds x to bf16 (rel err ~1e-3 RMS) and then
splits it EXACTLY into two fp8-e4m3 halves, x = hi + lo: hi is the e4m3
rounding of x (top 4 mantissa bits) and lo = x - hi is an integer multiple
of the bf16 ulp no larger than 8 ulps, hence also exact in e4m3.  T's small
integers are exact in e4m3 too, so the device computes T@hi + T@lo in two
fp8 DoubleRow matmuls (K=256 packed two-per-partition, 0.5 PE cycles/row --
half the PE time of the bf16 equivalent) accumulating exactly into fp32
PSUM.  Output is evicted PSUM->SBUF with a bf16 cast and DMA'd out as bf16;
the host upcasts to fp32.  Total error ~2e-3, all from the initial bf16
rounding of x and the final bf16 rounding of y.

HBM traffic per core: 3.2MB in (two e4m3 tensors) + 6.4MB out (bf16)
+ 0.26MB weights ~= 9.9MB, half the fp32-I/O figure.  Work is spread
across all five engine queues (PE matmuls + a few output DMAs; SP input
DMAs + output DMAs; Pool copies + output DMAs; DVE/ACT PSUM-evict copies)
so no single engine holds more than ~15us of the ~74us total.
"""

import os
import sys

import numpy as np

for _p in ("/opt/trn_rl_repo", "/root/.axon_site/_ro/trn_rl_repo"):
    if os.path.isdir(_p) and _p not in sys.path:
        sys.path.append(_p)

B, C_IN, C_OUT, HH, WW = 64, 256, 512, 28, 28
S = HH * WW  # 784
N_CORES = 8
BS = B // N_CORES  # 8 samples per core
N_PASSES, GROUPS = 8, 8

VARIANT = os.environ.get("DWHT_VARIANT", "fp8dr")

# spatial split per PSUM bank (each chunk <= 512 fp32 = one bank)
N_CHUNKS = ((0, 392), (392, 392))


def _dwht_T() -> np.ndarray:
    """Build the (512, 256) transform matrix by running the reference
    butterfly (including its partial-update in-place semantics) on identity."""
    x = np.zeros((C_OUT, C_IN), np.float64)
    x[:C_IN] = np.eye(C_IN)
    half = C_OUT // 2
    for _ in range(N_PASSES):
        top = x[::2] + x[1::2]
        x = x.copy()
        x[:half] = top
        bottom = x[::2] - x[1::2]
        x[half:] = bottom
    # channel shuffle with groups=8
    x = x.reshape(GROUPS, C_OUT // GROUPS, C_IN).transpose(1, 0, 2).reshape(C_OUT, C_IN)
    return x


def _plan(env, default):
    return os.environ.get(env, default).split(",")


def _interleave(counts):
    """Bresenham-interleave engine names by target counts -> flat list."""
    total = sum(counts.values())
    acc = {k: 0.0 for k in counts}
    out = []
    for i in range(total):
        k = max(counts, key=lambda k: counts[k] / total * (i + 1) - acc[k])
        acc[k] += 1
        out.append(k)
    return out


def _build(variant):
    import concourse.mybir as mybir
    from concourse import bacc
    from concourse.tile import TileContext

    f32 = mybir.dt.float32
    bf16 = mybir.dt.bfloat16
    f8 = mybir.dt.float8e4

    fp8 = variant == "fp8dr"
    perf_mode = mybir.MatmulPerfMode.DoubleRow if fp8 else None

    # Engine schedules.  Inputs are front-loaded on the two HWDGE queues so
    # the PE never starves; early outputs go to Pool/ACT (SP is busy with
    # inputs), late outputs to SP (inputs done); copies are spread
    # Pool-heavy (327ns each) with DVE next (533ns) and ACT last (552ns).
    def sched(env, default):
        p = _plan(env, default)
        return lambda i: p[i % len(p)]

    # inputs: xh on scalar, xl on sync (sync also carries the weights first)
    # -- both HWDGE queues feed in parallel the whole run, finishing ~5us in,
    # after which both take output DMAs.
    in_s = sched("DWHT_IN_PLAN", "scalar,sync")
    # full-tile outs (28, samples 0-6) alternate the two HWDGE queues with a
    # couple on Pool; Pool is otherwise the copy workhorse (327ns/copy vs
    # DVE 533 / ACT 552)
    # NOTE: Pool (gpsimd) tensor_copy does not lower on the axon backend --
    # copies stay on DVE/ACT; Pool carries output DMAs instead.
    out_s = sched("DWHT_OUT_PLAN",
                  "gpsimd,sync,gpsimd,scalar,sync,gpsimd,sync,gpsimd")
    copy_s = sched("DWHT_COPY_PLAN", "vector,scalar")
    # last-sample drain: per-chunk DMAs fanned across both HWDGE queues
    tail_dma = sched("DWHT_TAIL_DMA", "scalar,sync")
    nwarm = int(os.environ.get("DWHT_WARM", "0"))
    prefetch = int(os.environ.get("DWHT_PREFETCH", "4"))
    xs_bufs = int(os.environ.get("DWHT_XS_BUFS", "12"))

    nc = bacc.Bacc(None, target_bir_lowering=False)
    if fp8:
        # host-packed [s, p, (t a), f]: t = hi/lo split half, a = k-half,
        # p = partition -- one contiguous 3136B run per partition per sample
        xhl = nc.dram_tensor("xhl", (BS, 128, 4, S), f8, kind="ExternalInput")
        tt = nc.dram_tensor("tt", (C_IN, C_OUT), f8, kind="ExternalInput")
    else:
        x = nc.dram_tensor("x", (BS, C_IN, S), bf16, kind="ExternalInput")
        tt = nc.dram_tensor("tt", (C_IN, C_OUT), bf16, kind="ExternalInput")
    y = nc.dram_tensor("y", (BS, C_OUT, S), bf16, kind="ExternalOutput")

    def eng(name):
        return getattr(nc, name)

    def copy_op(name, dst, src):
        if name == "scalar":
            nc.scalar.copy(dst, src)
        else:
            eng(name).tensor_copy(dst, src)

    with TileContext(nc) as tc:
        with (
            tc.tile_pool(name="w", bufs=1) as wp,
            tc.tile_pool(name="io", bufs=3) as io,
            tc.tile_pool(name="ps", bufs=8, space="PSUM") as pp,
        ):
            if fp8:
                # [128, 2, 512]: partition p, k-half a, out-channel m.
                # Split per m-tile so the first matmul's weights land ASAP
                # behind the fixed HWDGE pipe-init latency.
                tw = wp.tile([128, 2, C_OUT], f8, tag="tw")
                ttr = tt.rearrange("(a p) m -> p a m", p=128)
                nc.sync.dma_start(out=tw[:], in_=ttr)
                if nwarm:
                    # dummy matmuls on the weight tile to ramp the PE clock
                    # while the first input DMAs are in flight
                    warm = pp.tile([128, 392], f32, tag="ps", name="warm")
                    for wi in range(nwarm):
                        nc.tensor.matmul(
                            warm[:],
                            tw[:, :, 0:128],
                            tw[:, :, 0:392],
                            start=(wi == 0),
                            stop=(wi == nwarm - 1),
                            perf_mode=perf_mode,
                        )
            else:
                tts = []
                for k in range(2):
                    t = wp.tile([128, C_OUT], bf16, tag=f"tt{k}")
                    nc.sync.dma_start(out=t[:], in_=tt[k * 128 : (k + 1) * 128, :])
                    tts.append(t)

            n_out = n_cp = 0
            in_tiles = {}

            def issue_in(s):
                if fp8:
                    xt = io.tile([128, 4, S], f8, tag="xs", bufs=xs_bufs)
                    if s == 0:
                        # split by half and chunk across idle queues:
                        # minimize time-to-first-matmul behind DMA pipe-init
                        for ti, q in ((0, "scalar"), (1, "gpsimd")):
                            tsl = slice(2 * ti, 2 * ti + 2)
                            for n0, nsz in N_CHUNKS:
                                eng(q).dma_start(
                                    out=xt[:, tsl, n0 : n0 + nsz],
                                    in_=xhl[s][:, tsl, n0 : n0 + nsz],
                                )
                    else:
                        eng(in_s(s)).dma_start(out=xt[:], in_=xhl[s])
                    srcs = [xt[:, 0:2], xt[:, 2:4]]
                else:
                    srcs = []
                    for k in range(2):
                        xt = io.tile([128, S], bf16, tag="xs", bufs=xs_bufs)
                        eng(in_s(2 * s + k)).dma_start(
                            out=xt[:], in_=x[s, k * 128 : (k + 1) * 128, :]
                        )
                        srcs.append(xt)
                in_tiles[s] = srcs

            for s in range(min(prefetch, BS)):
                issue_in(s)

            for s in range(BS):
                last = s == BS - 1
                if s + prefetch < BS:
                    issue_in(s + prefetch)
                srcs = in_tiles.pop(s)

                for m in range(C_OUT // 128):
                    msl = slice(m * 128, (m + 1) * 128)
                    ysm = io.tile([128, S], bf16, tag="ysm", bufs=12,
                                  name="ysm")
                    for ni, (n0, nsz) in enumerate(N_CHUNKS):
                        nsl = slice(n0, n0 + nsz)
                        ps = pp.tile([128, nsz], f32, tag="ps")
                        if fp8:
                            for i, xt in enumerate(srcs):
                                nc.tensor.matmul(
                                    ps[:],
                                    tw[:, :, msl],
                                    xt[:, :, nsl],
                                    start=(i == 0),
                                    stop=(i == 1),
                                    perf_mode=perf_mode,
                                )
                        else:
                            for ki in range(2):
                                nc.tensor.matmul(
                                    ps[:],
                                    tts[ki][:, msl],
                                    srcs[ki][:, nsl],
                                    start=(ki == 0),
                                    stop=(ki == 1),
                                )
                        ci = m * len(N_CHUNKS) + ni
                        copy_op(copy_s(n_cp), ysm[:, nsl], ps[:])
                        n_cp += 1
                        if last:
                            # drain per-chunk on fanned-out queues
                            eng(tail_dma(ci)).dma_start(
                                out=y[s, msl, nsl], in_=ysm[:, nsl]
                            )
                    if not last:
                        eng(out_s(n_out)).dma_start(
                            out=y[s, msl, :], in_=ysm[:]
                        )
                    n_out += 1

    nc.compile()
    return nc


_cache = {}


def _get_nc(variant, reps=1):
    key = (
        variant,
        os.environ.get("DWHT_IN_PLAN"),
        os.environ.get("DWHT_OUT_PLAN"),
        os.environ.get("DWHT_COPY_PLAN"),
    )
    if key not in _cache:
        _cache[key] = _build(variant)
    return _cache[key]


def _in_maps(x_np, variant):
    import ml_dtypes

    T = _dwht_T()
    ttT = np.ascontiguousarray(T.T)  # (256, 512), lhsT layout
    if variant == "fp8dr":
        tt_np = ttT.astype(ml_dtypes.float8_e4m3fn)
        x_bf = x_np.astype(ml_dtypes.bfloat16).astype(np.float32)
        x_hi = x_bf.astype(ml_dtypes.float8_e4m3fn)
        x_lo = (x_bf - x_hi.astype(np.float32)).astype(ml_dtypes.float8_e4m3fn)
        # pack [b, p, (t a), f]: j=0 hi k<128, j=1 hi k>=128, j=2/3 lo halves
        xhl = np.empty((B, 128, 4, S), ml_dtypes.float8_e4m3fn)
        xhl[:, :, 0] = x_hi[:, 0:128]
        xhl[:, :, 1] = x_hi[:, 128:256]
        xhl[:, :, 2] = x_lo[:, 0:128]
        xhl[:, :, 3] = x_lo[:, 128:256]
        return [
            {"xhl": xhl[i * BS : (i + 1) * BS], "tt": tt_np}
            for i in range(N_CORES)
        ]
    tt_np = ttT.astype(ml_dtypes.bfloat16)
    x_bf = x_np.astype(ml_dtypes.bfloat16)
    return [
        {"x": x_bf[i * BS : (i + 1) * BS], "tt": tt_np} for i in range(N_CORES)
    ]


def _run(x_np, variant=None, trace=False, reps=1):
    from concourse.bass_utils import run_bass_kernel_spmd

    variant = variant or VARIANT
    nc = _get_nc(variant, reps)
    res = run_bass_kernel_spmd(
        nc, _in_maps(x_np, variant), list(range(N_CORES)), trace=trace
    )
    y = (
        np.stack([np.asarray(r["y"], np.float32) for r in res.results])
        .reshape(B, C_OUT, HH, WW)
    )
    return y, res


def kernel(x: np.ndarray) -> np.ndarray:
    x_np = np.ascontiguousarray(np.asarray(x), dtype=np.float32).reshape(B, C_IN, S)
    y, _ = _run(x_np)
    return y
